# revision 1
# baseline (speedup 1.0000x reference)
"""AttentionDTI on 8 Trainium2 NeuronCores — pure data-parallel over batch.

Strategy
--------
B=8 batches -> 1 batch per core (SPMD, no collectives). All parameters are
replicated; tokens are sharded along batch. The reference materializes the
(B, 85, 979, 160) pairwise tensor in HBM and applies a 160x160 linear to every
grid cell; since mean() commutes with the linear map, we only ever need
  Sd[i, c]  = sum_j relu(d_att[i, c] + p_att[j, c])     (row sums)
  Sp[j, c]  = sum_i relu(d_att[i, c] + p_att[j, c])     (col sums)
computed tile-by-tile in SBUF (the grid never touches HBM), followed by the
Wa linear + sigmoid on the tiny (85+979, 160) results.

Grid: channels live on partitions (chunk c0 = 0:128, c1 = 128:160 packed four
i-values per tile in 22-column blocks), protein positions on the free axis.
Each "unit" is one relu(p_att + d_att[i]) tile, produced on the Scalar engine
(activation with per-partition bias + free-axis accum -> Sd column for free),
the Vector engine (scalar_tensor_tensor with accum), or GPSIMD — split by a
weighted round-robin to balance busy time — while the Tensor engine
accumulates Sp via identity-matmul into PSUM.

All conv / attention-linear matmuls run in bf16 (fp32 matmul is 4 cycles/row
on the PE; bf16 is 1). PSUM accumulation stays fp32. Small parameters are
packed host-side into a handful of row-grouped blobs so the whole kernel
issues ~15 DMAs (HWDGE descriptor generation is ~0.6us per DMA, serialized);
token DMAs go first, the big MLP weights last.
"""

import os
import sys

import numpy as np

for _p in ("/opt/trn_rl_repo", "/root/.axon_site/_ro/trn_rl_repo"):
    if os.path.isdir(_p) and _p not in sys.path:
        sys.path.append(_p)

import concourse.bass as bass  # noqa: E402,F401
import concourse.bacc as bacc  # noqa: E402
import concourse.mybir as mybir  # noqa: E402
import concourse.tile as tile  # noqa: E402
from concourse import bass_utils  # noqa: E402

AFT = mybir.ActivationFunctionType
ALU = mybir.AluOpType
DT = mybir.dt
F32 = DT.float32
I32 = DT.int32
AXX = mybir.AxisListType.X

NCORES = 8
B, LD, LP, DIM, CV = 8, 100, 1000, 64, 40
C = 4 * CV  # 160
DL1, DL2, DL3 = 97, 92, 85  # drug lengths after k=4,6,8 valid convs
PL1, PL2, PL3 = 997, 990, 979  # protein lengths after k=4,8,12
NEG = -1.0e9  # bias for padded i-slots: relu(p + NEG) == 0
NGRP = (DL3 + 3) // 4  # 22 packed groups for channels 128:160 (block layout)

R32 = DT.bfloat16  # PE operand dtype: 1 cycle/row. (float32r would
# match bf16 speed at fp32-read precision but trips walrus ISA checks
# in this toolchain; plain fp32 is 4 cycles/row => ~2.5x slower.)
GRID_DT = R32  # pairwise grid tiles are written pre-rounded for the PE
MM_DT = R32  # conv / attention-linear matmul operands likewise
# engine weights for grid units: (ACT, DVE, GPSIMD) ~ per-engine unit rate
ENGINE_W = (0.90, 1.00, 0.00)  # GPSIMD can't run TensorScalarPtr (walrus engine check)

_TRACE = bool(int(os.environ.get("KERNEL_TRACE", "0")))
LAST_RESULT = None
_CACHE = {}

# (pack -> (rows, [(key, cols, to_bf16), ...]));  bf16 entries must be a
# contiguous prefix so one tensor_copy converts the whole region.
PACKS = {
    "pk128": (128, [("drug_emb", DIM, 1), ("prot_emb", DIM, 1),
                    ("ident", 128, 1), ("WdaT_c0", C, 1), ("WpaT_c0", C, 1),
                    ("WaT_c0", C, 1), ("ident4", 32, 1), ("db3_c0", 1, 0),
                    ("pb3_c0", 1, 0), ("bda_c0", 1, 0), ("bpa_c0", 1, 0),
                    ("ba_c0", 1, 0), ("iota", 1, 0)]),
    "pk80": (80, [("dW3T", 8 * C, 1), ("pW3T", 12 * C, 1),
                  ("db2", 1, 0), ("pb2", 1, 0)]),
    "pk64": (64, [("dW1T", 4 * CV, 1), ("pW1T", 4 * CV, 1)]),
    "pk40": (40, [("dW2T", 6 * 2 * CV, 1), ("pW2T", 8 * 2 * CV, 1),
                  ("db1", 1, 0), ("pb1", 1, 0)]),
    "pk32": (32, [("WdaT_c1", C, 1), ("WpaT_c1", C, 1), ("WaT_c1", C, 1),
                  ("db3_c1", 1, 0), ("pb3_c1", 1, 0), ("bda_c1", 1, 0),
                  ("bpa_c1", 1, 0), ("ba_c1", 1, 0)]),
    "pkrow": (1, [("fb1", 1024, 0), ("fb2", 1024, 0), ("fb3", 512, 0),
                  ("fb4", 1, 0)]),
}


def _pack_offsets(pack):
    rows, entries = PACKS[pack]
    off, out = 0, {}
    for key, cols, bf in entries:
        out[key] = (off, cols, bf)
        off += cols
    return rows, off, out


def _jtiles(n, step=512):
    return [(o, min(step, n - o)) for o in range(0, n, step)]


def _mchunks(n):
    return [(o, min(128, n - o)) for o in range(0, n, 128)]


# --------------------------------------------------------------------------
# host-side parameter packing (pure marshalling, replicated to all cores)
# --------------------------------------------------------------------------

def _prep_shared(inp):
    def f32(x):
        return np.ascontiguousarray(np.asarray(x), dtype=np.float32)

    def convT(w):  # (co, ci, k) -> (ci, k*co) with [:, k0*co:(k0+1)*co] = tap k0
        co, ci, k = w.shape
        return np.ascontiguousarray(f32(w).transpose(1, 2, 0).reshape(ci, k * co))

    WdaT, WpaT, WaT = f32(inp["Wda"]).T, f32(inp["Wpa"]).T, f32(inp["Wa"]).T
    src = {
        "ident": np.eye(128, dtype=np.float32),
        "ident4": np.tile(np.eye(32, dtype=np.float32), (4, 1)),
        "iota": np.arange(128, dtype=np.float32).reshape(128, 1),
        "drug_emb": f32(inp["drug_emb"]), "prot_emb": f32(inp["prot_emb"]),
        "dW1T": convT(inp["dW1"]), "dW2T": convT(inp["dW2"]), "dW3T": convT(inp["dW3"]),
        "pW1T": convT(inp["pW1"]), "pW2T": convT(inp["pW2"]), "pW3T": convT(inp["pW3"]),
        "WdaT_c0": WdaT[0:128], "WdaT_c1": WdaT[128:C],
        "WpaT_c0": WpaT[0:128], "WpaT_c1": WpaT[128:C],
        "WaT_c0": WaT[0:128], "WaT_c1": WaT[128:C],
        "db1": f32(inp["db1"]).reshape(-1, 1), "db2": f32(inp["db2"]).reshape(-1, 1),
        "db3_c0": f32(inp["db3"]).reshape(-1, 1)[0:128],
        "db3_c1": f32(inp["db3"]).reshape(-1, 1)[128:C],
        "pb1": f32(inp["pb1"]).reshape(-1, 1), "pb2": f32(inp["pb2"]).reshape(-1, 1),
        "pb3_c0": f32(inp["pb3"]).reshape(-1, 1)[0:128],
        "pb3_c1": f32(inp["pb3"]).reshape(-1, 1)[128:C],
        "bda_c0": f32(inp["bda"]).reshape(-1, 1)[0:128],
        "bda_c1": f32(inp["bda"]).reshape(-1, 1)[128:C],
        "bpa_c0": f32(inp["bpa"]).reshape(-1, 1)[0:128],
        "bpa_c1": f32(inp["bpa"]).reshape(-1, 1)[128:C],
        "ba_c0": f32(inp["ba"]).reshape(-1, 1)[0:128],
        "ba_c1": f32(inp["ba"]).reshape(-1, 1)[128:C],
        "fb1": f32(inp["fb1"]).reshape(1, -1), "fb2": f32(inp["fb2"]).reshape(1, -1),
        "fb3": f32(inp["fb3"]).reshape(1, -1), "fb4": f32(inp["fb4"]).reshape(1, -1),
    }
    d = {}
    for pack in PACKS:
        rows, tot, offs = _pack_offsets(pack)
        arr = np.zeros((rows, tot), np.float32)
        for key, (off, cols, _) in offs.items():
            a = src[key]
            arr[0:a.shape[0], off:off + cols] = a
        d[pack] = arr
    # fW1T packed as 4 column segments (128/32/128/32 rows)
    fW1T = f32(inp["fW1"]).T  # (320, 1024)
    f1 = np.zeros((128, 4096), np.float32)
    for s, (r0, rn) in enumerate([(0, 128), (128, 32), (160, 128), (288, 32)]):
        f1[0:rn, 1024 * s:1024 * (s + 1)] = fW1T[r0:r0 + rn]
    d["fW1P"] = f1

    def pmajor(wT):  # (K, M) -> (128, (K/128)*M): chunk c at cols [c*M:(c+1)*M]
        k, m = wT.shape
        return np.ascontiguousarray(
            wT.reshape(k // 128, 128, m).transpose(1, 0, 2).reshape(128, -1))

    d["fW2P"] = pmajor(f32(inp["fW2"]).T)
    d["fW3P"] = pmajor(f32(inp["fW3"]).T)
    d["fW4P"] = pmajor(f32(inp["fW4"]).T)
    return d


def _dram_specs():
    specs = {"drug": ((1, LD), I32), "protein": ((1, LP), I32)}
    for pack in PACKS:
        rows, tot, _ = _pack_offsets(pack)
        specs[pack] = ((rows, tot), F32)
    specs["fW1P"] = ((128, 4096), F32)
    specs["fW2P"] = ((128, 8192), F32)
    specs["fW3P"] = ((128, 4096), F32)
    specs["fW4P"] = ((128, 4), F32)
    return specs


_DRAM_SPECS = _dram_specs()


# --------------------------------------------------------------------------
# device kernel
# --------------------------------------------------------------------------

def build(grid_dt=GRID_DT, engine_w=ENGINE_W, debug=False, opts=()):
    opts = set(opts)
    nc = bacc.Bacc("TRN2", target_bir_lowering=False, debug=debug,
                   num_devices=NCORES)
    dram = {}
    for name, (shape, dt_) in _DRAM_SPECS.items():
        dram[name] = nc.dram_tensor(name, list(shape), dt_,
                                    kind="ExternalInput").ap()
    out_dram = nc.dram_tensor("out", [1, 1], F32, kind="ExternalOutput").ap()

    with tile.TileContext(nc) as tc:
        with (
            tc.tile_pool(name="w", bufs=1) as wp,
            tc.tile_pool(name="s", bufs=1) as sp,
            tc.tile_pool(name="h", bufs=4) as hp,
            tc.tile_pool(name="ps", bufs=3, space="PSUM") as pp,
            tc.tile_pool(name="pg", bufs=1, space="PSUM") as pg,
        ):
            _body(nc, tc, wp, sp, hp, pp, pg, dram, out_dram, grid_dt,
                  engine_w, opts)
    nc.compile()
    return nc


def _body(nc, tc, wp, sp, hp, pp, pg, dram, out_dram, grid_dt, engine_w, opts):
    # ---- token DMAs first (broadcast to vocab rows in the DMA itself) ---
    tokb_p = sp.tile([26, LP], I32, tag="tokb_p")
    nc.sync.dma_start(tokb_p, dram["protein"].broadcast_to((26, LP)))
    tokb_d = sp.tile([65, LD], I32, tag="tokb_d")
    nc.sync.dma_start(tokb_d, dram["drug"].broadcast_to((65, LD)))

    # ---- packed parameter loads; matmul-consumed regions (the bf-marked
    # prefix of each pack) are rounded to f32r by a DVE copy -------------
    pk_f32, pk_r, pk_rcols = {}, {}, {}
    for pack in PACKS:
        rows, tot, offs = _pack_offsets(pack)
        t = wp.tile([rows, tot], F32, tag=pack)
        nc.sync.dma_start(t, dram[pack])
        pk_f32[pack] = t
        rcols = sum(cols for _, (off, cols, bf) in offs.items() if bf)
        pk_rcols[pack] = rcols
        if rcols:
            pk_r[pack] = wp.tile([rows, rcols], R32,
                                 name=f"{pack}_r", tag=f"{pack}_r")

    def convert(pack, c0=0, c1=None):
        c1 = pk_rcols[pack] if c1 is None else c1
        nc.vector.tensor_copy(pk_r[pack][:, c0:c1], pk_f32[pack][:, c0:c1])

    convert("pk128", 0, 2 * DIM)  # embedding tables first: shortest dep chain

    def r32(ap):
        return ap if ap.dtype == R32 else ap.bitcast(R32)

    def P(key, rows=None, bf=True):
        for pack in PACKS:
            prows, _, offs = _pack_offsets(pack)
            if key in offs:
                off, cols, isbf = offs[key]
                t = pk_r[pack] if (bf and isbf) else pk_f32[pack]
                return t[0:(rows or prows), off:off + cols]
        raise KeyError(key)

    ones1 = wp.tile([1, 1], F32, tag="ones1")
    nc.vector.memset(ones1, 1.0)
    # warm the sigmoid ACT-table set now (relu/copy/identity are in every
    # set, so no further table loads happen mid-kernel)
    actwarm = wp.tile([1, 1], F32, tag="actwarm")
    nc.scalar.activation(actwarm, ones1, AFT.Sigmoid)
    ident = P("ident")
    ident4 = P("ident4")

    # ---- embeddings via one-hot matmul --------------------------------
    def embed(tokb, vocab, length, emb_sb, name):
        oh = sp.tile([vocab, length], MM_DT, tag=f"oh_{name}")
        nc.vector.tensor_scalar(oh, tokb, P("iota", rows=vocab, bf=False),
                                None, ALU.is_equal)
        res = sp.tile([DIM, length], MM_DT, tag=f"e_{name}")
        for j0, jn in _jtiles(length):
            ps = pp.tile([DIM, jn], F32, tag="ps")
            nc.tensor.matmul(ps, r32(emb_sb), r32(oh[:, j0:j0 + jn]),
                             start=True, stop=True)
            nc.scalar.activation(res[:, j0:j0 + jn], ps, AFT.Copy)
        return res

    pe = embed(tokb_p, 26, LP, P("prot_emb", rows=26), "p")
    de = embed(tokb_d, 65, LD, P("drug_emb", rows=65), "d")

    convert("pk64")
    convert("pk40")
    convert("pk80")
    convert("pk128", 2 * DIM)
    convert("pk32")
    zeros = wp.tile([128, PL3], GRID_DT, tag="zeros")
    nc.vector.memset(zeros, 0.0)

    # ---- CNN stacks (conv as K shifted matmuls accumulated in PSUM) ----
    def conv(tag, x, wT, biases, cout, k, lout):
        outs = []
        n_ev = 0
        for ci, (mo, msz) in enumerate(_mchunks(cout)):
            o = sp.tile([msz, lout], MM_DT, tag=f"{tag}_{mo}")
            outs.append(o)
            for j0, jn in _jtiles(lout):
                ps = pp.tile([msz, jn], F32, tag="ps")
                for t in range(k):
                    nc.tensor.matmul(ps, r32(wT[:, cout * t + mo: cout * t + mo + msz]),
                                     r32(x[:, j0 + t: j0 + t + jn]),
                                     start=(t == 0), stop=(t == k - 1))
                if n_ev % 2 == 0:
                    nc.scalar.activation(o[:, j0:j0 + jn], ps, AFT.Relu,
                                         bias=biases[ci])
                else:
                    nc.vector.scalar_tensor_tensor(o[:, j0:j0 + jn], ps,
                                                   biases[ci], zeros[0:msz, 0:jn],
                                                   ALU.add, ALU.max)
                n_ev += 1
        return outs

    pc1 = conv("pc1", pe, P("pW1T", rows=DIM), [P("pb1", bf=False)], CV, 4, PL1)[0]
    dc1 = conv("dc1", de, P("dW1T", rows=DIM), [P("db1", bf=False)], CV, 4, DL1)[0]
    pc2 = conv("pc2", pc1, P("pW2T"), [P("pb2", bf=False)], 2 * CV, 8, PL2)[0]
    dc2 = conv("dc2", dc1, P("dW2T"), [P("db2", bf=False)], 2 * CV, 6, DL2)[0]
    pc3 = conv("pc3", pc2, P("pW3T"), [P("pb3_c0", bf=False), P("pb3_c1", bf=False)],
               C, 12, PL3)
    dc3 = conv("dc3", dc2, P("dW3T"), [P("db3_c0", bf=False), P("db3_c1", bf=False)],
               C, 8, DL3)

    # ---- attention linears --------------------------------------------
    def att_linear(tag, wTk, biases, xs, length, out_dt, act=AFT.Identity,
                   scale=1.0, pad_cols=0, order=(0, 1)):
        outs = [None, None]
        for ci in order:
            mo, msz = _mchunks(C)[ci]
            o = sp.tile([msz, length + pad_cols], out_dt, tag=f"{tag}_{mo}")
            if pad_cols:
                nc.vector.memset(o[:, length:length + pad_cols], NEG)
            for j0, jn in _jtiles(length):
                ps = pp.tile([msz, jn], F32, tag="ps")
                for kc in range(len(xs)):
                    nc.tensor.matmul(ps, r32(wTk[kc][:, mo:mo + msz]),
                                     r32(xs[kc][:, j0:j0 + jn]),
                                     start=(kc == 0), stop=(kc == len(xs) - 1))
                nc.scalar.activation(o[:, j0:j0 + jn], ps, act, bias=biases[ci],
                                     scale=scale)
            outs[ci] = o
        return outs

    WpaT = [P("WpaT_c0"), P("WpaT_c1")]
    WdaT = [P("WdaT_c0"), P("WdaT_c1")]
    WaT = [P("WaT_c0"), P("WaT_c1")]
    bpac = [P("bpa_c0", bf=False), P("bpa_c1", bf=False)]
    bdac = [P("bda_c0", bf=False), P("bda_c1", bf=False)]
    bac = [P("ba_c0", bf=False), P("ba_c1", bf=False)]

    datt = att_linear("datt", WdaT, bdac, dc3, DL3, F32, pad_cols=3, order=(1, 0))
    patt = att_linear("patt", WpaT, bpac, pc3, PL3, grid_dt, order=(1, 0))

    # block-packed per-partition bias for channels 128:160:
    #   dattb_pk[32a + p, g] = datt_b[p, 22a + g]   (i = 22a + g, 85..87 = NEG)
    dattb_pk = sp.tile([128, NGRP], F32, tag="dattb_pk")
    for a in range(4):
        nc.sync.dma_start(dattb_pk[32 * a:32 * a + 32, :],
                          datt[1][:, NGRP * a:NGRP * a + NGRP])
    # protein c1 chunk replicated x4 on partitions
    patt_b4 = sp.tile([128, PL3], grid_dt, tag="patt_b4")
    for a in range(4):
        nc.sync.dma_start(patt_b4[32 * a:32 * a + 32, :], patt[1])

    # ---- the pairwise grid --------------------------------------------
    sd_c0 = sp.tile([128, DL3], F32, tag="sd_c0")
    sd_pk = sp.tile([128, NGRP], F32, tag="sd_pk")
    sp_a = pg.tile([128, PL3], F32, tag="sp_a", padded_shape=[128, 1024])
    sp_b = pg.tile([32, PL3], F32, tag="sp_b", padded_shape=[32, 1024])

    # a dozen c0 units lead so the c1 bias-pack/replicate DMAs overlap them;
    # then all c1 units so their Sp/Sd consumers overlap the c0 bulk.
    NLEAD = 12
    units = ([("c0", i) for i in range(NLEAD)]
             + [("c1", g) for g in range(NGRP)]
             + [("c0", i) for i in range(NLEAD, DL3)])
    if "no_grid" in opts:
        units = [("c1", 0), ("c0", 0)]
    first_q = {k: min(q for k2, q in units if k2 == k) for k in ("c0", "c1")}
    last_q = {k: max(q for k2, q in units if k2 == k) for k in ("c0", "c1")}

    deficits = [0.0, 0.0, 0.0]
    engines = [
        lambda h, src, b_ap, s_ap: nc.scalar.activation(
            h, src, AFT.Relu, bias=b_ap, accum_out=s_ap),
        lambda h, src, b_ap, s_ap: nc.vector.scalar_tensor_tensor(
            h, src, b_ap, zeros, ALU.add, ALU.max, accum_out=s_ap),
        lambda h, src, b_ap, s_ap: nc.gpsimd.scalar_tensor_tensor(
            h, src, b_ap, zeros, ALU.add, ALU.max, accum_out=s_ap),
    ]

    def emit_unit(kind, q):
        c0 = kind == "c0"
        src = patt[0] if c0 else patt_b4
        bias_ap = (datt[0] if c0 else dattb_pk)[:, q:q + 1]
        sd_ap = (sd_c0 if c0 else sd_pk)[:, q:q + 1]
        h = hp.tile([128, PL3], grid_dt, tag="H")
        for e in range(3):
            deficits[e] += engine_w[e]
        e = max(range(3), key=lambda i: deficits[i])
        deficits[e] -= sum(engine_w)
        engines[e](h, src, bias_ap, sd_ap)
        lhs = ident if c0 else ident4
        pst = sp_a if c0 else sp_b
        first = q == first_q[kind]
        last = q == last_q[kind]
        for j0, jn in _jtiles(PL3):
            nc.tensor.matmul(pst[:, j0:j0 + jn], r32(lhs), r32(h[:, j0:j0 + jn]),
                             start=first, stop=last)

    n_head = max(i for i, (k, _) in enumerate(units) if k == "c1") + 1
    for kind, q in units[:n_head]:
        emit_unit(kind, q)

    # c1 done: its Sp/Sd consumers can overlap the c0 portion of the grid.
    # Block-packed Sd unpacks with 4 contiguous DMAs: sd_b[:, 22a+g] = sd_pk[32a+p, g].
    spb_sb = sp.tile([32, PL3], R32, tag="spb_sb")
    for j0, jn in _jtiles(PL3):
        nc.scalar.activation(spb_sb[:, j0:j0 + jn], sp_b[:, j0:j0 + jn], AFT.Copy)
    sd_b = sp.tile([32, NGRP * 4], F32, tag="sd_b")
    for a in range(4):
        nc.sync.dma_start(sd_b[:, NGRP * a:NGRP * a + NGRP],
                          sd_pk[32 * a:32 * a + 32, :])
    sd_bm = sp.tile([32, DL3], R32, tag="sd_bm")
    nc.vector.tensor_copy(sd_bm, sd_b[:, 0:DL3])

    for kind, q in units[n_head:]:
        emit_unit(kind, q)

    spa_sb = sp.tile([128, PL3], R32, tag="spa_sb")
    for j0, jn in _jtiles(PL3):
        nc.scalar.activation(spa_sb[:, j0:j0 + jn], sp_a[:, j0:j0 + jn], AFT.Copy)
    sd_c0m = sp.tile([128, DL3], R32, tag="sd_c0m")
    nc.vector.tensor_copy(sd_c0m, sd_c0)

    # ---- attention outputs: sigmoid(Wa @ mean + ba) -------------------
    # k-chunk c1 emitted first in each group: its operand is ready mid-grid.
    catt = att_linear("catt", [WaT[1], WaT[0]], bac, [sd_bm, sd_c0m], DL3,
                      F32, act=AFT.Sigmoid, scale=1.0 / PL3)
    # ---- protein tail: per (jtile, chunk): Wa matmuls -> sigmoid -> gate
    # -> partial max, fully pipelined across engines ----------------------
    jts = _jtiles(PL3)
    pvv = [sp.tile([msz, len(jts)], F32, name=f"pvv_{mo}", tag=f"pvv_{mo}")
           for mo, msz in _mchunks(C)]
    for t, (j0, jn) in enumerate(jts):
        for ci, (mo, msz) in enumerate(_mchunks(C)):
            ps = pp.tile([msz, jn], F32, tag="ps")
            nc.tensor.matmul(ps, r32(WaT[1][:, mo:mo + msz]),
                             r32(spb_sb[:, j0:j0 + jn]), start=True, stop=False)
            nc.tensor.matmul(ps, r32(WaT[0][:, mo:mo + msz]),
                             r32(spa_sb[:, j0:j0 + jn]), start=False, stop=True)
            pr = sp.tile([msz, jn], F32, name=f"pr_{mo}_{t}", tag="prt", bufs=2)
            nc.scalar.activation(pr, ps, AFT.Sigmoid, bias=bac[ci],
                                 scale=1.0 / DL3)
            g = sp.tile([msz, jn], F32, name=f"gp_{mo}_{t}", tag="gpt", bufs=2)
            nc.vector.scalar_tensor_tensor(g, pr, 0.5, pc3[ci][:, j0:j0 + jn],
                                           ALU.add, ALU.mult)
            nc.vector.tensor_reduce(pvv[ci][:, t:t + 1], g, AXX, ALU.max)
    pv = []
    for ci, (mo, msz) in enumerate(_mchunks(C)):
        v = sp.tile([msz, 1], F32, tag=f"pv_{mo}")
        nc.vector.tensor_reduce(v, pvv[ci], AXX, ALU.max)
        pv.append(v)

    # drug side is tiny: single-tile gate + max
    dv = []
    for ci, (mo, msz) in enumerate(_mchunks(C)):
        g = sp.tile([msz, DL3], F32, tag=f"gd_{mo}")
        nc.vector.scalar_tensor_tensor(g, catt[ci][:, 0:DL3], 0.5, dc3[ci],
                                       ALU.add, ALU.mult)
        v = sp.tile([msz, 1], F32, tag=f"dv_{mo}")
        nc.vector.tensor_reduce(v, g, AXX, ALU.max)
        dv.append(v)

    # ---- final MLP (weights DMA'd last; m on partitions, n=1 matvecs) --
    def wide_load(name, nchunks):
        shape, _ = _DRAM_SPECS[name]
        cols = shape[1] // nchunks
        t = wp.tile([128, shape[1]], F32, tag=name)
        nc.sync.dma_start(t, dram[name])
        return [t[:, cols * j:cols * (j + 1)] for j in range(nchunks)]

    fW1t = wp.tile([128, 4096], F32, tag="fW1P")
    nc.sync.dma_start(fW1t, dram["fW1P"])
    fW1k = [fW1t[0:128, 0:1024], fW1t[0:32, 1024:2048],
            fW1t[0:128, 2048:3072], fW1t[0:32, 3072:4096]]
    fW2k = wide_load("fW2P", 8)
    fW3k = wide_load("fW3P", 8)
    fW4k = wide_load("fW4P", 4)
    fb1, fb2, fb3 = P("fb1"), P("fb2"), P("fb3")
    fb4 = P("fb4")

    def dense(tag, xk, wk, bias_row, m, leaky):
        nm = m // 128
        ps = pp.tile([128, nm], F32, tag="ps")
        for mc in range(nm):
            for ci, (xv, wt) in enumerate(zip(xk, wk)):
                nc.tensor.matmul(ps[:, mc:mc + 1], wt[:, 128 * mc:128 * mc + 128],
                                 xv, start=(ci == 0), stop=False)
            nc.tensor.matmul(ps[:, mc:mc + 1], bias_row[0:1, 128 * mc:128 * mc + 128],
                             ones1, start=False, stop=True)
        yr = sp.tile([128, nm], F32, tag=f"yr{tag}")
        nc.vector.tensor_copy(yr, ps)
        if not leaky:
            return yr
        y = sp.tile([128, nm], F32, tag=f"y{tag}")
        nc.vector.scalar_tensor_tensor(y, yr, 0.01, yr, ALU.mult, ALU.max)
        return y

    y1 = dense("1", [dv[0], dv[1], pv[0], pv[1]], fW1k, fb1, 1024, True)
    y2 = dense("2", [y1[:, j:j + 1] for j in range(8)], fW2k, fb2, 1024, True)
    y3 = dense("3", [y2[:, j:j + 1] for j in range(8)], fW3k, fb3, 512, True)

    y4ps = pp.tile([1, 1], F32, tag="ps")
    for ci in range(4):
        nc.tensor.matmul(y4ps, fW4k[ci], y3[:, ci:ci + 1],
                         start=(ci == 0), stop=False)
    nc.tensor.matmul(y4ps, fb4, ones1, start=False, stop=True)
    res = sp.tile([1, 1], F32, tag="res")
    nc.vector.tensor_copy(res, y4ps)
    nc.sync.dma_start(out_dram, res)


# --------------------------------------------------------------------------
# entry point
# --------------------------------------------------------------------------

def _get_nc():
    key = (GRID_DT, ENGINE_W)
    if key not in _CACHE:
        _CACHE[key] = build()
    return _CACHE[key]


def kernel(**inputs):
    global LAST_RESULT
    nc = _get_nc()
    shared = _prep_shared(inputs)
    drug = np.ascontiguousarray(np.asarray(inputs["drug"]), dtype=np.int32)
    protein = np.ascontiguousarray(np.asarray(inputs["protein"]), dtype=np.int32)
    in_maps = []
    for b in range(NCORES):
        m = dict(shared)
        m["drug"] = drug[b:b + 1]
        m["protein"] = protein[b:b + 1]
        in_maps.append(m)
    res = bass_utils.run_bass_kernel_spmd(nc, in_maps, core_ids=list(range(NCORES)),
                                          trace=_TRACE)
    LAST_RESULT = res
    out = np.concatenate([res.results[b]["out"] for b in range(NCORES)], axis=0)
    return out.astype(np.float32)



# revision 49
# speedup vs baseline: 1.8545x; 1.8545x over previous
"""AttentionDTI on 8 Trainium2 NeuronCores — pure data-parallel over batch.

Strategy
--------
B=8 batches -> 1 batch per core (SPMD, no collectives). All parameters are
replicated; tokens are sharded along batch. The reference materializes the
(B, 85, 979, 160) pairwise tensor in HBM and applies a 160x160 linear to every
grid cell; since mean() commutes with the linear map, we only ever need
  Sd[i, c]  = sum_j relu(d_att[i, c] + p_att[j, c])     (row sums)
  Sp[j, c]  = sum_i relu(d_att[i, c] + p_att[j, c])     (col sums)
computed tile-by-tile in SBUF (the grid never touches HBM), followed by the
Wa linear + sigmoid on the tiny (85+979, 160) results.

Grid: channels live on partitions (chunk c0 = 0:128, c1 = 128:160 packed four
i-values per tile in 22-column blocks), protein positions on the free axis.
Each "unit" is one relu(p_att + d_att[i]) tile, produced on the Scalar engine
(activation with per-partition bias + free-axis accum -> Sd column for free),
the Vector engine (scalar_tensor_tensor with accum), or GPSIMD — split by a
weighted round-robin to balance busy time — while the Tensor engine
accumulates Sp via identity-matmul into PSUM.

All conv / attention-linear matmuls run in bf16 (fp32 matmul is 4 cycles/row
on the PE; bf16 is 1). PSUM accumulation stays fp32. Small parameters are
packed host-side into a handful of row-grouped blobs so the whole kernel
issues ~15 DMAs (HWDGE descriptor generation is ~0.6us per DMA, serialized);
token DMAs go first, the big MLP weights last.
"""

import os
import sys

import numpy as np

for _p in ("/opt/trn_rl_repo", "/root/.axon_site/_ro/trn_rl_repo"):
    if os.path.isdir(_p) and _p not in sys.path:
        sys.path.append(_p)

import concourse.bass as bass  # noqa: E402,F401
import concourse.bacc as bacc  # noqa: E402
import concourse.mybir as mybir  # noqa: E402
import concourse.tile as tile  # noqa: E402
from concourse import bass_utils  # noqa: E402

AFT = mybir.ActivationFunctionType
ALU = mybir.AluOpType
DT = mybir.dt
F32 = DT.float32
I32 = DT.int32
AXX = mybir.AxisListType.X

NCORES = 8
B, LD, LP, DIM, CV = 8, 100, 1000, 64, 40
C = 4 * CV  # 160
DL1, DL2, DL3 = 97, 92, 85  # drug lengths after k=4,6,8 valid convs
PL1, PL2, PL3 = 997, 990, 979  # protein lengths after k=4,8,12
NEG = -1.0e9  # bias for padded i-slots: relu(p + NEG) == 0
NGRP = (DL3 + 3) // 4  # 22 packed groups for channels 128:160 (block layout)

R32 = DT.bfloat16  # PE operand dtype: 1 cycle/row. (float32r would
# match bf16 speed at fp32-read precision but trips walrus ISA checks
# in this toolchain; plain fp32 is 4 cycles/row => ~2.5x slower.)
GRID_DT = R32  # pairwise grid tiles are written pre-rounded for the PE
MM_DT = R32  # conv / attention-linear matmul operands likewise
# engine weights for grid units: (ACT, DVE, GPSIMD) ~ per-engine unit rate.
# Measured per 979-col unit: ACT ~1.32us (activation + accumulator read),
# DVE STT at 2x bf16 ~0.93us.
ENGINE_W = (0.70, 1.00, 0.00)  # GPSIMD can't run TensorScalarPtr (walrus engine check)
N_WARM = 20  # dummy matmuls at t=0: HAM clock-gate warmup (1.2 -> 2.4 GHz)

_TRACE = bool(int(os.environ.get("KERNEL_TRACE", "0")))
LAST_RESULT = None
_CACHE = {}

# (pack -> (rows, [(key, cols, to_bf16), ...]));  bf16 entries must be a
# contiguous prefix so one tensor_copy converts the whole region.
# All conv / attention matmuls are zero-padded to K=128 contraction rows
# (and conv outputs to M=128 columns): padding costs no PE cycles (cost is
# N-bound) but keeps the systolic array fully occupied — the HAM clock
# governor re-throttles the PE to half clock when it sees partial-array
# activity, and once re-throttled mid-kernel it tends to stick.
# Packs are merged into 4 DMAs ordered by when their consumers run —
# HWDGE descriptor generation is ~0.6us per DMA and strictly serial on the
# sync queue, so the startup-critical tensors must be in the first few.
PACKS = {
    # (rows, dtype, entries). Weight packs ship pre-rounded bf16 from the
    # host: half the HBM bytes and no on-device convert pass at all. The
    # tiny f32 pack carries per-partition biases + the iota column.
    "pkbias": (128, "f32", [("iota", 1, 0), ("db3_c0", 1, 0),
                            ("pb3_c0", 1, 0), ("db3_c1", 1, 0),
                            ("pb3_c1", 1, 0), ("bda_c0", 1, 0),
                            ("bpa_c0", 1, 0), ("ba_c0", 1, 0),
                            ("db1", 1, 0), ("pb1", 1, 0), ("db2", 1, 0),
                            ("pb2", 1, 0), ("bda_c1", 1, 0),
                            ("bpa_c1", 1, 0), ("ba_c1", 1, 0)]),
    "pkemb": (128, "bf16", [("drug_emb", 128, 1), ("prot_emb", 128, 1)]),
    "pke": (128, "bf16", [("ident", 128, 1), ("WdaT_c0", C, 1),
                          ("WpaT_c0", C, 1), ("WaT_c0", C, 1),
                          ("ident4", 32, 1), ("WdaT_c1", C, 1),
                          ("WpaT_c1", C, 1), ("dW1T", 4 * 128, 1),
                          ("pW1T", 4 * 128, 1)]),
    "pkm": (128, "bf16", [("dW2T", 6 * 128, 1), ("pW2T", 8 * 128, 1),
                          ("WaT_c1", C, 1)]),
    "pk80": (128, "bf16", [("pW3T", 12 * 256, 1), ("dW3T", 8 * 256, 1)]),
    "pkrow": (1, "bf16", [("fb1", 1024, 0), ("fb2", 1024, 0),
                          ("fb3", 512, 0), ("fb4", 1, 0)]),
}


def _pack_offsets(pack):
    rows, _, entries = PACKS[pack]
    off, out = 0, {}
    for key, cols, bf in entries:
        out[key] = (off, cols, bf)
        off += cols
    return rows, off, out


def _jtiles(n, step=512):
    return [(o, min(step, n - o)) for o in range(0, n, step)]


def _mchunks(n):
    return [(o, min(128, n - o)) for o in range(0, n, 128)]


# --------------------------------------------------------------------------
# host-side parameter packing (pure marshalling, replicated to all cores)
# --------------------------------------------------------------------------

def _prep_shared(inp):
    def f32(x):
        return np.ascontiguousarray(np.asarray(x), dtype=np.float32)

    def padT(w, mpad):  # (co, ci, k) -> (128, k*mpad): tap t block at t*mpad,
        # zero-padded to K=128 rows and M=mpad cols per tap
        co, ci, k = w.shape
        arr = np.zeros((128, k * mpad), np.float32)
        wf = f32(w)
        for t in range(k):
            arr[0:ci, t * mpad:t * mpad + co] = wf[:, :, t].T
        return arr

    WdaT, WpaT, WaT = f32(inp["Wda"]).T, f32(inp["Wpa"]).T, f32(inp["Wa"]).T
    src = {
        "ident": np.eye(128, dtype=np.float32),
        "ident4": np.tile(np.eye(32, dtype=np.float32), (4, 1)),
        "iota": np.arange(128, dtype=np.float32).reshape(128, 1),
        "drug_emb": f32(inp["drug_emb"]), "prot_emb": f32(inp["prot_emb"]),
        "dW1T": padT(inp["dW1"], 128), "dW2T": padT(inp["dW2"], 128),
        "dW3T": padT(inp["dW3"], 256),
        "pW1T": padT(inp["pW1"], 128), "pW2T": padT(inp["pW2"], 128),
        "pW3T": padT(inp["pW3"], 256),
        "WdaT_c0": WdaT[0:128], "WdaT_c1": WdaT[128:C],
        "WpaT_c0": WpaT[0:128], "WpaT_c1": WpaT[128:C],
        "WaT_c0": WaT[0:128], "WaT_c1": WaT[128:C],
        "db1": f32(inp["db1"]).reshape(-1, 1), "db2": f32(inp["db2"]).reshape(-1, 1),
        "db3_c0": f32(inp["db3"]).reshape(-1, 1)[0:128],
        "db3_c1": f32(inp["db3"]).reshape(-1, 1)[128:C],
        "pb1": f32(inp["pb1"]).reshape(-1, 1), "pb2": f32(inp["pb2"]).reshape(-1, 1),
        "pb3_c0": f32(inp["pb3"]).reshape(-1, 1)[0:128],
        "pb3_c1": f32(inp["pb3"]).reshape(-1, 1)[128:C],
        "bda_c0": f32(inp["bda"]).reshape(-1, 1)[0:128],
        "bda_c1": f32(inp["bda"]).reshape(-1, 1)[128:C],
        "bpa_c0": f32(inp["bpa"]).reshape(-1, 1)[0:128],
        "bpa_c1": f32(inp["bpa"]).reshape(-1, 1)[128:C],
        "ba_c0": f32(inp["ba"]).reshape(-1, 1)[0:128],
        "ba_c1": f32(inp["ba"]).reshape(-1, 1)[128:C],
        "fb1": f32(inp["fb1"]).reshape(1, -1), "fb2": f32(inp["fb2"]).reshape(1, -1),
        "fb3": f32(inp["fb3"]).reshape(1, -1), "fb4": f32(inp["fb4"]).reshape(1, -1),
    }
    import ml_dtypes
    bf16 = ml_dtypes.bfloat16
    d = {}
    for pack, (rows, dt_, entries) in PACKS.items():
        _, tot, offs = _pack_offsets(pack)
        arr = np.zeros((rows, tot), np.float32)
        for key, (off, cols, _) in offs.items():
            a = src[key]
            arr[0:a.shape[0], off:off + a.shape[1]] = a  # zero-pad the rest
        d[pack] = arr.astype(bf16) if dt_ == "bf16" else arr
    # fW1T packed as 4 column segments (128/32/128/32 rows); MLP weights ship
    # as bf16 (halves HBM traffic + enables fast weight load on the PE)
    fW1T = f32(inp["fW1"]).T  # (320, 1024)
    f1 = np.zeros((128, 4096), np.float32)
    for s, (r0, rn) in enumerate([(0, 128), (128, 32), (160, 128), (288, 32)]):
        f1[0:rn, 1024 * s:1024 * (s + 1)] = fW1T[r0:r0 + rn]
    d["fW1P"] = f1.astype(bf16)

    def pmajor(wT):  # (K, M) -> (128, (K/128)*M): chunk c at cols [c*M:(c+1)*M]
        k, m = wT.shape
        return np.ascontiguousarray(
            wT.reshape(k // 128, 128, m).transpose(1, 0, 2).reshape(128, -1)).astype(bf16)

    d["fW2P"] = pmajor(f32(inp["fW2"]).T)
    d["fW3P"] = pmajor(f32(inp["fW3"]).T)
    d["fW4P"] = pmajor(f32(inp["fW4"]).T)
    return d


def _dram_specs():
    specs = {"drug": ((1, LD), I32), "protein": ((1, LP), I32)}
    for pack, (rows, dt_, entries) in PACKS.items():
        _, tot, _ = _pack_offsets(pack)
        specs[pack] = ((rows, tot), R32 if dt_ == "bf16" else F32)
    specs["fW1P"] = ((128, 4096), R32)
    specs["fW2P"] = ((128, 8192), R32)
    specs["fW3P"] = ((128, 4096), R32)
    specs["fW4P"] = ((128, 4), R32)
    return specs


_DRAM_SPECS = _dram_specs()


# --------------------------------------------------------------------------
# device kernel
# --------------------------------------------------------------------------

def build(grid_dt=GRID_DT, engine_w=ENGINE_W, debug=False, opts=()):
    opts = set(opts)
    nc = bacc.Bacc("TRN2", target_bir_lowering=False, debug=debug,
                   num_devices=NCORES)
    dram = {}
    for name, (shape, dt_) in _DRAM_SPECS.items():
        dram[name] = nc.dram_tensor(name, list(shape), dt_,
                                    kind="ExternalInput").ap()
    out_dram = nc.dram_tensor("out", [1, 1], F32, kind="ExternalOutput").ap()

    with tile.TileContext(nc) as tc:
        with (
            tc.tile_pool(name="w", bufs=1) as wp,
            tc.tile_pool(name="s", bufs=1) as sp,
            tc.tile_pool(name="h", bufs=4) as hp,
            tc.tile_pool(name="ps", bufs=3, space="PSUM") as pp,
            tc.tile_pool(name="pg", bufs=1, space="PSUM") as pg,
        ):
            _body(nc, tc, wp, sp, hp, pp, pg, dram, out_dram, grid_dt,
                  engine_w, opts)
    nc.compile()
    return nc


def _body(nc, tc, wp, sp, hp, pp, pg, dram, out_dram, grid_dt, engine_w, opts):
    # ---- token DMAs first (broadcast to vocab rows in the DMA itself) ---
    tokb_p = sp.tile([26, LP], I32, tag="tokb_p")
    nc.sync.dma_start(tokb_p, dram["protein"].broadcast_to((26, LP)))
    tokb_d = sp.tile([65, LD], I32, tag="tokb_d")
    nc.sync.dma_start(tokb_d, dram["drug"].broadcast_to((65, LD)))

    # ---- PE clock-gate management: the HAM throttles the PE to K=4/8
    # (half clock) and only sustained activity releases it; a ~5us idle gap
    # re-throttles — and once re-throttled mid-kernel it has been observed
    # stuck cold for 50us+. Dummy "spin" matmuls on junk data keep the PE
    # busy across known dependency stalls (startup DMA wait, embed->conv
    # handoff, grid->tail PSUM drain) so real matmuls always run warm.
    wjunk = sp.tile([128, 512], R32, tag="wjunk")
    nc.vector.memset(wjunk, 0.0)
    wps = pp.tile([128, 512], F32, tag="warmps", bufs=1)
    spin_state = [0]

    def spin(n, tether=None):
        # Alternate disjoint PSUM halves so consecutive spins pipeline
        # (same-region WAW would serialize on the full array drain).
        # `tether` anchors the spins in the schedule right after the tile
        # they (meaninglessly) read — Tile orders engine programs by deps,
        # so untethered spins get hoisted to kernel start.
        for _ in range(n):
            half = spin_state[0] % 2
            spin_state[0] += 1
            if tether is None:
                lhs, rows, m = wjunk[:, 0:128], 128, 128
            else:
                t = tether if tether.dtype == R32 else tether.bitcast(R32)
                rows = min(128, t.partition_size())
                m = min(128, t.free_size())
                lhs = t[0:rows, 0:m]
            nc.tensor.matmul(wps[0:m, 256 * half:256 * half + 256],
                             lhs, wjunk[0:rows, 0:256], start=True,
                             stop=True, skip_group_check=True)

    spin(N_WARM)

    # ---- packed parameter loads; matmul-consumed regions (the bf-marked
    # prefix of each pack) are rounded to f32r by a DVE copy -------------
    pk_t = {}
    for pack, (rows, dt_, entries) in PACKS.items():
        _, tot, offs = _pack_offsets(pack)
        t = wp.tile([rows, tot], R32 if dt_ == "bf16" else F32, tag=pack)
        if pack != "pkrow":  # pkrow (MLP biases) is DMA'd with the MLP weights
            nc.sync.dma_start(t, dram[pack])
        pk_t[pack] = t

    def r32(ap):
        return ap if ap.dtype == R32 else ap.bitcast(R32)

    def P(key, rows=None, bf=True):
        for pack in PACKS:
            prows, _, offs = _pack_offsets(pack)
            if key in offs:
                off, cols, _ = offs[key]
                return pk_t[pack][0:(rows or prows), off:off + cols]
        raise KeyError(key)

    ones1 = wp.tile([1, 1], R32, tag="ones1")
    nc.vector.memset(ones1, 1.0)
    ident = P("ident")
    ident4 = P("ident4")

    # ---- embeddings via one-hot matmul --------------------------------
    def embed(tokb, vocab, length, emb_sb, name):
        # one-hot rows padded to 128 (tail memset once): the embed matmul
        # then runs at full K=128 array occupancy for free
        oh = sp.tile([128, length], MM_DT, tag=f"oh_{name}")
        nc.vector.memset(oh, 0.0)  # no deps: runs before the tokens land
        nc.vector.tensor_scalar(oh[0:vocab, :], tokb,
                                P("iota", bf=False)[0:vocab, :],
                                None, ALU.is_equal)
        res = sp.tile([128, length], MM_DT, tag=f"e_{name}")
        for j0, jn in _jtiles(length):
            ps = pp.tile([128, jn], F32, tag="ps")
            nc.tensor.matmul(ps, r32(emb_sb), r32(oh[:, j0:j0 + jn]),
                             start=True, stop=True)
            nc.scalar.activation(res[:, j0:j0 + jn], ps, AFT.Copy)
        return res

    pe = embed(tokb_p, 26, LP, P("prot_emb"), "p")
    de = embed(tokb_d, 65, LD, P("drug_emb"), "d")
    # warm the sigmoid ACT-table set now (it also holds relu/copy/identity,
    # covering every ACT op until the MLP's lrelu); write into wjunk — which
    # the spins read — so the warm op isn't dead-code-eliminated
    nc.scalar.activation(wjunk[0:1, 0:2].bitcast(F32), ones1, AFT.Sigmoid)
    spin(12, tether=pe)  # embed->conv handoff

    zeros = wp.tile([128, PL3], GRID_DT, tag="zeros")
    nc.vector.memset(zeros, 0.0)

    # ---- CNN stacks (conv as K shifted matmuls accumulated in PSUM) ----
    def conv(tag, x, wT, biases, nchunks, k, lout):
        # x: [128, L] (real rows on top, zero rows below); wT: K/M-padded
        # (128, k * nchunks * 128). Padded rows/cols contribute exact zeros,
        # so outputs land zero-extended to 128 partitions.
        outs = []
        n_ev = 0
        mstride = nchunks * 128
        for ci in range(nchunks):
            o = sp.tile([128, lout], MM_DT, tag=f"{tag}_{ci}")
            outs.append(o)
            for j0, jn in _jtiles(lout):
                ps = pp.tile([128, jn], F32, tag="ps")
                for t in range(k):
                    off = mstride * t + 128 * ci
                    nc.tensor.matmul(ps, r32(wT[:, off:off + 128]),
                                     r32(x[:, j0 + t: j0 + t + jn]),
                                     start=(t == 0), stop=(t == k - 1))
                if n_ev % 2 == 0:
                    nc.scalar.activation(o[:, j0:j0 + jn], ps, AFT.Relu,
                                         bias=biases[ci])
                else:
                    nc.vector.scalar_tensor_tensor(o[:, j0:j0 + jn], ps,
                                                   biases[ci], zeros[:, 0:jn],
                                                   ALU.add, ALU.max)
                n_ev += 1
        return outs

    pc1 = conv("pc1", pe, P("pW1T"), [P("pb1", bf=False)], 1, 4, PL1)[0]
    dc1 = conv("dc1", de, P("dW1T"), [P("db1", bf=False)], 1, 4, DL1)[0]
    pc2 = conv("pc2", pc1, P("pW2T"), [P("pb2", bf=False)], 1, 8, PL2)[0]
    dc2 = conv("dc2", dc1, P("dW2T"), [P("db2", bf=False)], 1, 6, DL2)[0]
    pc3 = conv("pc3", pc2, P("pW3T"), [P("pb3_c0", bf=False), P("pb3_c1", bf=False)],
               2, 12, PL3)
    dc3 = conv("dc3", dc2, P("dW3T"), [P("db3_c0", bf=False), P("db3_c1", bf=False)],
               2, 8, DL3)

    # ---- attention linears --------------------------------------------
    def att_linear(tag, wTk, biases, xs, length, out_dt, act=AFT.Identity,
                   scale=1.0, pad_cols=0, order=(0, 1)):
        outs = [None, None]
        for ci in order:
            mo, msz = _mchunks(C)[ci]
            o = sp.tile([msz, length + pad_cols], out_dt, tag=f"{tag}_{mo}")
            if pad_cols:
                nc.vector.memset(o[:, length:length + pad_cols], NEG)
            for j0, jn in _jtiles(length):
                ps = pp.tile([msz, jn], F32, tag="ps")
                for kc in range(len(xs)):
                    nc.tensor.matmul(ps, r32(wTk[kc][:, mo:mo + msz]),
                                     r32(xs[kc][:, j0:j0 + jn]),
                                     start=(kc == 0), stop=(kc == len(xs) - 1))
                nc.scalar.activation(o[:, j0:j0 + jn], ps, act, bias=biases[ci],
                                     scale=scale)
            outs[ci] = o
        return outs

    WpaT = [P("WpaT_c0"), P("WpaT_c1")]
    WdaT = [P("WdaT_c0"), P("WdaT_c1")]
    WaT = [P("WaT_c0"), P("WaT_c1", rows=32)]
    bpac = [P("bpa_c0", bf=False), P("bpa_c1", rows=32, bf=False)]
    bdac = [P("bda_c0", bf=False), P("bda_c1", rows=32, bf=False)]
    bac = [P("ba_c0", bf=False), P("ba_c1", rows=32, bf=False)]

    spin(2, tether=dc3[1])  # conv->attention handoff
    spin(2, tether=pc3[1])
    datt = att_linear("datt", WdaT, bdac, dc3, DL3, F32, pad_cols=3, order=(1, 0))
    spin(2, tether=datt[0])
    patt = att_linear("patt", WpaT, bpac, pc3, PL3, grid_dt, order=(1, 0))
    spin(2, tether=patt[1])

    # block-packed per-partition bias for channels 128:160:
    #   dattb_pk[32a + p, g] = datt_b[p, 22a + g]   (i = 22a + g, 85..87 = NEG)
    dattb_pk = sp.tile([128, NGRP], F32, tag="dattb_pk")
    for a in range(4):
        nc.sync.dma_start(dattb_pk[32 * a:32 * a + 32, :],
                          datt[1][:, NGRP * a:NGRP * a + NGRP])
    # protein c1 chunk replicated x4 on partitions
    patt_b4 = sp.tile([128, PL3], grid_dt, tag="patt_b4")
    for a in range(4):
        nc.sync.dma_start(patt_b4[32 * a:32 * a + 32, :], patt[1])

    # ---- the pairwise grid --------------------------------------------
    sd_c0 = sp.tile([128, DL3], F32, tag="sd_c0")
    sd_pk = sp.tile([128, NGRP], F32, tag="sd_pk")
    sp_a = pg.tile([128, PL3], F32, tag="sp_a", padded_shape=[128, 1024])
    sp_b = pg.tile([32, PL3], F32, tag="sp_b", padded_shape=[32, 1024])

    # a dozen c0 units lead so the c1 bias-pack/replicate DMAs overlap them;
    # then all c1 units so their Sp/Sd consumers overlap the c0 bulk.
    NLEAD = 12
    units = ([("c0", i) for i in range(NLEAD)]
             + [("c1", g) for g in range(NGRP)]
             + [("c0", i) for i in range(NLEAD, DL3)])
    if "no_grid" in opts:
        units = [("c1", 0), ("c0", 0)]
    first_q = {k: min(q for k2, q in units if k2 == k) for k in ("c0", "c1")}
    last_q = {k: max(q for k2, q in units if k2 == k) for k in ("c0", "c1")}

    deficits = [0.0, 0.0, 0.0]
    # NOTE: tensor_scalar(+accum_out) would hit the DVE 4x packed mode, but
    # it returns corrupted results on TRN2 hardware (probed: both the
    # elementwise output and the accumulator are garbage). STT is exact.
    engines = [
        lambda h, src, b_ap, s_ap: nc.scalar.activation(
            h, src, AFT.Relu, bias=b_ap, accum_out=s_ap),
        lambda h, src, b_ap, s_ap: nc.vector.scalar_tensor_tensor(
            h, src, b_ap, zeros, ALU.add, ALU.max, accum_out=s_ap),
        lambda h, src, b_ap, s_ap: nc.gpsimd.scalar_tensor_tensor(
            h, src, b_ap, zeros, ALU.add, ALU.max, accum_out=s_ap),
    ]

    def emit_unit(kind, q):
        c0 = kind == "c0"
        src = patt[0] if c0 else patt_b4
        bias_ap = (datt[0] if c0 else dattb_pk)[:, q:q + 1]
        sd_ap = (sd_c0 if c0 else sd_pk)[:, q:q + 1]
        h = hp.tile([128, PL3], grid_dt, tag="H")
        for e in range(3):
            deficits[e] += engine_w[e]
        e = max(range(3), key=lambda i: deficits[i])
        deficits[e] -= sum(engine_w)
        engines[e](h, src, bias_ap, sd_ap)
        emit_unit.last_h = h
        lhs = ident if c0 else ident4
        pst = sp_a if c0 else sp_b
        first = q == first_q[kind]
        last = q == last_q[kind]
        for j0, jn in _jtiles(PL3):
            nc.tensor.matmul(pst[:, j0:j0 + jn], r32(lhs), r32(h[:, j0:j0 + jn]),
                             start=first, stop=last)

    n_head = max(i for i, (k, _) in enumerate(units) if k == "c1") + 1
    for kind, q in units[:n_head]:
        emit_unit(kind, q)

    # c1 done: its Sp/Sd consumers can overlap the c0 portion of the grid.
    # Block-packed Sd unpacks with 4 contiguous DMAs: sd_b[:, 22a+g] = sd_pk[32a+p, g].
    spb_sb = sp.tile([32, PL3], R32, tag="spb_sb")
    for j0, jn in _jtiles(PL3):  # (GPSIMD can't read PSUM; ACT does this)
        nc.scalar.activation(spb_sb[:, j0:j0 + jn], sp_b[:, j0:j0 + jn], AFT.Copy)
    sd_b = sp.tile([32, NGRP * 4], F32, tag="sd_b")
    for a in range(4):
        nc.sync.dma_start(sd_b[:, NGRP * a:NGRP * a + NGRP],
                          sd_pk[32 * a:32 * a + 32, :])
    sd_bm = sp.tile([32, DL3], R32, tag="sd_bm")
    nc.vector.tensor_copy(sd_bm, sd_b[:, 0:DL3])

    tail_units = units[n_head:]
    for ui, (kind, q) in enumerate(tail_units):
        emit_unit(kind, q)
        # HAM insurance: the produce-bound grid runs the PE at ~60-70% duty
        # and its MID window re-throttles near the end; a spin every few
        # units keeps it at K=8/8 into the tail.
        if (ui >= len(tail_units) - 24 and ui % 4 == 0) or (
                ui >= len(tail_units) - 12 and ui % 2 == 0):
            spin(1, tether=emit_unit.last_h)
    spin(4, tether=sd_c0)  # grid->tail: keep the PE warm across the Sp drain

    sd_c0m = sp.tile([128, DL3], R32, tag="sd_c0m")
    nc.vector.tensor_copy(sd_c0m, sd_c0)
    spin(2, tether=sd_c0m)

    # ---- protein tail: per (jtile, chunk): Sp PSUM->SBUF drain, then Wa
    # matmuls -> sigmoid -> gate -> partial max, pipelined across engines --
    spa_sb = sp.tile([128, PL3], R32, tag="spa_sb")
    jts = _jtiles(PL3)
    # catt first: its matmuls run during the Sp-drain stall, and its sigmoid
    # chain (-> drug gates -> dv -> xb) completes while the Wa tail streams
    catt = att_linear("catt", [WaT[1], WaT[0]], bac, [sd_bm, sd_c0m], DL3,
                      F32, act=AFT.Sigmoid, scale=1.0 / PL3)
    # drain Sp PSUM->SBUF with ACT and DVE in parallel (one jtile each)
    nc.scalar.activation(spa_sb[:, 0:jts[0][1]], sp_a[:, 0:jts[0][1]], AFT.Copy)
    nc.vector.tensor_copy(spa_sb[:, jts[1][0]:jts[1][0] + jts[1][1]],
                          sp_a[:, jts[1][0]:jts[1][0] + jts[1][1]])
    spin(4, tether=spa_sb[0:128, 0:128])
    # drug side is tiny: single-tile gate + max; emitted before the protein
    # tail so the dv -> xb chain completes while the Wa matmuls stream
    dv = []
    for ci, (mo, msz) in enumerate(_mchunks(C)):
        gdt = sp.tile([msz, DL3], F32, tag=f"gd_{mo}")
        nc.vector.scalar_tensor_tensor(gdt, catt[ci][:, 0:DL3], 0.5,
                                       dc3[ci][0:msz, :],
                                       ALU.add, ALU.mult)
        v = sp.tile([msz, 1], F32, tag=f"dv_{mo}")
        nc.vector.tensor_reduce(v, gdt, AXX, ALU.max)
        dv.append(v)
    xb = []
    for i, v in enumerate(dv):
        b = sp.tile([_mchunks(C)[i % 2][1], 1], R32, tag=f"xb{i}")
        nc.vector.tensor_copy(b, v)
        xb.append(b)
    pvv = [sp.tile([msz, len(jts)], F32, name=f"pvv_{mo}", tag=f"pvv_{mo}")
           for mo, msz in _mchunks(C)]
    for t, (j0, jn) in enumerate(jts):
        for ci, (mo, msz) in enumerate(_mchunks(C)):
            ps = pp.tile([msz, jn], F32, tag="ps")
            nc.tensor.matmul(ps, r32(WaT[1][:, mo:mo + msz]),
                             r32(spb_sb[:, j0:j0 + jn]), start=True, stop=False)
            nc.tensor.matmul(ps, r32(WaT[0][:, mo:mo + msz]),
                             r32(spa_sb[:, j0:j0 + jn]), start=False, stop=True)
            pr = sp.tile([msz, jn], F32, name=f"pr_{mo}_{t}", tag="prt", bufs=2)
            nc.scalar.activation(pr, ps, AFT.Sigmoid, bias=bac[ci],
                                 scale=1.0 / DL3)
            last_pr = pr
            g = sp.tile([msz, jn], F32, name=f"gp_{mo}_{t}", tag="gpt", bufs=2)
            nc.vector.scalar_tensor_tensor(g, pr, 0.5,
                                           pc3[ci][0:msz, j0:j0 + jn],
                                           ALU.add, ALU.mult)
            nc.vector.tensor_reduce(pvv[ci][:, t:t + 1], g, AXX, ALU.max)
    # pre-load the lrelu ACT-table set (not in the sigmoid set): reads the
    # last pr tile so the scheduler orders it after the final sigmoid (the
    # sets would ping-pong otherwise), and the ~1.3us load overlaps the MLP
    # weight matmuls instead of blocking y1
    nc.scalar.activation(wjunk[0:1, 2:4].bitcast(F32), last_pr[0:1, 0:1],
                         AFT.Lrelu)

    pv = []
    for ci, (mo, msz) in enumerate(_mchunks(C)):
        v = sp.tile([msz, 1], F32, tag=f"pv_{mo}")
        nc.vector.tensor_reduce(v, pvv[ci], AXX, ALU.max)
        pv.append(v)

    # ---- final MLP (bf16 weights: half the HBM bytes, fast weight load;
    # m on partitions, n=1 matvecs; PSUM accumulation stays fp32) --------
    def wide_load(name, nchunks):
        shape, _ = _DRAM_SPECS[name]
        cols = shape[1] // nchunks
        t = wp.tile([128, shape[1]], R32, tag=name)
        nc.sync.dma_start(t, dram[name])
        return [t[:, cols * j:cols * (j + 1)] for j in range(nchunks)]

    nc.sync.dma_start(pk_t["pkrow"], dram["pkrow"])
    fW1t = wp.tile([128, 4096], R32, tag="fW1P")
    nc.sync.dma_start(fW1t, dram["fW1P"])
    fW1k = [fW1t[0:128, 0:1024], fW1t[0:32, 1024:2048],
            fW1t[0:128, 2048:3072], fW1t[0:32, 3072:4096]]
    fW2k = wide_load("fW2P", 8)
    fW3k = wide_load("fW3P", 8)
    fW4k = wide_load("fW4P", 4)
    fb1, fb2, fb3 = P("fb1"), P("fb2"), P("fb3")
    fb4 = P("fb4")

    for i, v in enumerate(pv):
        b = sp.tile([_mchunks(C)[i % 2][1], 1], R32, tag=f"xb{2 + i}")
        nc.vector.tensor_copy(b, v)
        xb.append(b)

    spin(3, tether=xb[0])  # cover the pooled-vector (DVE) wait before the MLP

    def dense(tag, xk, wk, bias_row, m, leaky):
        nm = m // 128
        ps = pp.tile([128, nm], F32, tag="ps")
        for mc in range(nm):
            for ci, (xv, wt) in enumerate(zip(xk, wk)):
                nc.tensor.matmul(ps[:, mc:mc + 1], wt[:, 128 * mc:128 * mc + 128],
                                 xv, start=(ci == 0), stop=False)
            nc.tensor.matmul(ps[:, mc:mc + 1], bias_row[0:1, 128 * mc:128 * mc + 128],
                             ones1, start=False, stop=True)
        y = sp.tile([128, nm], R32, tag=f"y{tag}")
        # Lrelu's table slope is exactly 0.01 (probed) == jax.nn.leaky_relu
        nc.scalar.activation(y, ps, AFT.Lrelu if leaky else AFT.Copy)
        return y

    y1 = dense("1", xb, fW1k, fb1, 1024, True)
    y2 = dense("2", [y1[:, j:j + 1] for j in range(8)], fW2k, fb2, 1024, True)
    y3 = dense("3", [y2[:, j:j + 1] for j in range(8)], fW3k, fb3, 512, True)

    y4ps = pp.tile([1, 1], F32, tag="ps")
    for ci in range(4):
        nc.tensor.matmul(y4ps, fW4k[ci], y3[:, ci:ci + 1],
                         start=(ci == 0), stop=False)
    nc.tensor.matmul(y4ps, fb4, ones1, start=False, stop=True)
    res = sp.tile([1, 1], F32, tag="res")
    nc.vector.tensor_copy(res, y4ps)
    nc.sync.dma_start(out_dram, res)


# --------------------------------------------------------------------------
# entry point
# --------------------------------------------------------------------------

def _get_nc():
    key = (GRID_DT, ENGINE_W)
    if key not in _CACHE:
        _CACHE[key] = build()
    return _CACHE[key]


def kernel(**inputs):
    global LAST_RESULT
    nc = _get_nc()
    shared = _prep_shared(inputs)
    drug = np.ascontiguousarray(np.asarray(inputs["drug"]), dtype=np.int32)
    protein = np.ascontiguousarray(np.asarray(inputs["protein"]), dtype=np.int32)
    in_maps = []
    for b in range(NCORES):
        m = dict(shared)
        m["drug"] = drug[b:b + 1]
        m["protein"] = protein[b:b + 1]
        in_maps.append(m)
    res = bass_utils.run_bass_kernel_spmd(nc, in_maps, core_ids=list(range(NCORES)),
                                          trace=_TRACE)
    LAST_RESULT = res
    out = np.concatenate([res.results[b]["out"] for b in range(NCORES)], axis=0)
    return out.astype(np.float32)

